# revision 47
# baseline (speedup 1.0000x reference)
"""CBAM attention Trainium2 kernel.

Full inputs: x [32, 256, 64, 64], w1 [16, 256], w2 [256, 16], ws [1, 2, 7, 7].
Data-parallel across 8 NeuronCores: 4 samples per core, weights replicated.

Per-core layout: channels on partitions (2 tiles of 128 per sample), spatial
(64*64=4096) along the free dim.  x is pre-cast to bf16 on the host so the
HBM loads/stores move half the bytes and need no casting DMA (plain HWDGE on
the otherwise idle SP queue); the result is widened back to f32 on the host,
which is numerically identical to the on-device casting the reference-checked
baseline did.  Channel attention stats run as 4x-mode tensor_scalar accums;
the channel max over 256 channels is a pair tensor_max plus a GPSIMD
partition_all_reduce (Pool engine).  The 7x7 conv is a hybrid: dy taps 0-2
come from a small im2col gathered straight from the padded feature planes
(one 42-partition matmul), dy taps 3-6 stream dx-shifted rows (feat_dx)
directly into 4 more PSUM-accumulated matmuls; conv weights are replicated
across all 128 output partitions so the sigmoid gate is born broadcast over
channels.  Emission is software-pipelined (stage-major phases + next-rep
load/pool hoisting) to keep the in-order engine queues dense.
"""

import numpy as np

B, C, H, W = 32, 256, 64, 64
NCORES = 8
B_LOC = B // NCORES          # 4 samples per core
MID = 16
HW = H * W                   # 4096
NT = C // 128                # 2 channel tiles
PW = 70                      # padded row width (W + 2*3)
PH = 70                      # padded rows (H + 2*3)
PROW = PW * PH               # 4900 padded plane size
FROW = PW * (PH + 1)         # plane + spare row: dx shifts read 6 past end
IROW = H * PW                # 4480 im2col row size
KI = 3                       # dy taps covered by the im2col matmul
QY = 8                       # output y-rows per conv matmul group
QF = QY * W                  # 512 free elems (one PSUM bank) per group

_cached_nc = None

# emission schedule: (stage, skew) in within-iteration emission order.
# stage i runs on flat-sample index (j - skew) at loop iteration j.
SCHED = [(0, 0), (1, 1), (2, 1), (3, 1), (4, 3), (5, 3)]


def _build(reps=1, debug_taps=False):
    from concourse import bass, bacc, tile, library_config, bass_isa
    import concourse.mybir as mybir

    F32 = mybir.dt.float32
    BF16 = mybir.dt.bfloat16
    AF = mybir.ActivationFunctionType
    ALU = mybir.AluOpType

    nc = bacc.Bacc("TRN2", target_bir_lowering=False, debug=False,
                   num_devices=NCORES)

    x_d = nc.dram_tensor("x", [B_LOC, C, H, W], BF16, kind="ExternalInput")
    ones_d = nc.dram_tensor("ones", [128, 1], BF16, kind="ExternalInput")
    w1t_d = nc.dram_tensor("w1t", [128, NT, MID], F32, kind="ExternalInput")
    w1ta_d = nc.dram_tensor("w1ta", [128, NT, MID], F32,
                            kind="ExternalInput")
    w2t_d = nc.dram_tensor("w2t", [MID, C], F32, kind="ExternalInput")
    # wc3: lhsT for dy 0..KI-1, partition (c, dx, dy); wcd: per-dy lhsT for
    # dy KI..6, partition (c, dx).  All columns replicated 128-wide.
    wc3_d = nc.dram_tensor("wc3", [2 * 7 * KI, 128], BF16,
                           kind="ExternalInput")
    wcd_d = nc.dram_tensor("wcd", [14, 7 - KI, 128], BF16,
                           kind="ExternalInput")
    out_d = nc.dram_tensor("out", [B_LOC, C, H, W], BF16,
                           kind="ExternalOutput")
    if debug_taps:
        dbg_stat_d = nc.dram_tensor("dbg_stat", [128, 24], F32,
                                    kind="ExternalOutput")
        dbg_mean_d = nc.dram_tensor("dbg_mean", [1, PW * (PH + 1)], BF16,
                                    kind="ExternalOutput")
        dbg_max_d = nc.dram_tensor("dbg_max", [2, PW * (PH + 1)], BF16,
                                   kind="ExternalOutput")
        dbg_fdx_d = nc.dram_tensor("dbg_fdx", [14, PW * PH], BF16,
                                   kind="ExternalOutput")
        dbg_imc_d = nc.dram_tensor("dbg_imc", [2 * 7 * KI, H * PW], BF16,
                                   kind="ExternalOutput")
        dbg_grep_d = nc.dram_tensor("dbg_grep", [128, 2, HW // 2], BF16,
                                    kind="ExternalOutput")

    with tile.TileContext(nc) as tc:
        with (
            tc.tile_pool(name="xs", bufs=1) as xpool,
            tc.tile_pool(name="work", bufs=1) as work,
            tc.tile_pool(name="grep", bufs=2) as gpool,
            tc.tile_pool(name="gps", bufs=2, space="PSUM") as gpspool,
            tc.tile_pool(name="p4", bufs=2, space="PSUM") as p4pool,
            tc.tile_pool(name="mlp", bufs=2, space="PSUM") as mlppool,
        ):
            nc.gpsimd.load_library(library_config.attn)
            # ---- constants ----
            ones = work.tile([128, 1], BF16, tag="ones")
            w1t = work.tile([128, NT, MID], F32, tag="w1t")
            w1ta = work.tile([128, NT, MID], F32, tag="w1ta")
            w2t = work.tile([MID, C], F32, tag="w2t")
            wc3 = work.tile([2 * 7 * KI, 128], BF16, tag="wc3")
            wcd = work.tile([14, 7 - KI, 128], BF16, tag="wcd")
            nc.sync.dma_start(ones[:], ones_d.ap())
            nc.sync.dma_start(w1t[:], w1t_d.ap())
            nc.sync.dma_start(w1ta[:], w1ta_d.ap())
            nc.sync.dma_start(w2t[:], w2t_d.ap())
            nc.sync.dma_start(wc3[:], wc3_d.ap())
            nc.sync.dma_start(wcd[:], wcd_d.ap())

            # ---- padded feature planes (guards zeroed once) ----
            NPAD = B_LOC if debug_taps else 2
            meanpads = [work.tile([1, FROW], BF16, tag=f"meanpad{i}",
                                  name=f"meanpad{i}") for i in range(NPAD)]
            maxpads = [work.tile([128, FROW], BF16, tag=f"maxpad{i}",
                                 name=f"maxpad{i}") for i in range(NPAD)]
            feat_dxs = [work.tile([14, PROW], BF16, tag=f"feat_dx{i}",
                                  name=f"feat_dx{i}") for i in range(B_LOC)]

            for i in range(NPAD):
                for pad in (meanpads[i], maxpads[i]):
                    pv = pad[:].rearrange("p (y x) -> p y x", y=PH + 1, x=PW)
                    nc.vector.memset(pv[0:1, 0:3, :], 0.0)
                    nc.vector.memset(pv[0:1, 67:71, :], 0.0)
                    nc.vector.memset(pv[0:1, 3:67, 0:3], 0.0)
                    nc.vector.memset(pv[0:1, 3:67, 67:70], 0.0)

            # ---- flat software pipeline over samples i = rep*B_LOC + b ----
            N = reps * B_LOC
            xbf_of, xm2_of, imc_of = {}, {}, {}
            stat_of, hs_of = {}, {}

            def S0_load(i):
                xbf_of[i] = xpool.tile([128, NT, HW], BF16, tag="xb",
                                       name=f"xb{i}",
                                       bufs=2 if debug_taps else 4)
                b = i % B_LOC
                src = x_d.ap()[b].rearrange("ch h w -> ch (h w)").rearrange(
                    "(t c) e -> c t e", t=NT, c=128)
                for t in range(NT):
                    nc.sync.dma_start(xbf_of[i][:, t:t + 1, :],
                                      src[:, t:t + 1, :])

            def S1_pool(i):
                rep, b = divmod(i, B_LOC)
                if b == 0:
                    # stat cols: 2*(t*4+b)=sum, +1=max ; 16+t*4+b = ca
                    stat_of[rep] = work.tile([128, 24], F32, tag="stat",
                                             bufs=2, name=f"stat{rep}")
                    hs_of[rep] = work.tile([MID, 3 * B_LOC], F32, tag="hs",
                                           bufs=2, name=f"hs{rep}")
                stat, xbf = stat_of[rep], xbf_of[i]
                for t in range(NT):
                    j = t * B_LOC + b
                    nc.vector.tensor_scalar(
                        xbf[:, t, :], xbf[:, t, :], 1.0, None,
                        op0=ALU.mult, op1=ALU.add,
                        accum_out=stat[:, 2 * j:2 * j + 1])
                    nc.vector.tensor_scalar(
                        xbf[:, t, :], xbf[:, t, :], 1.0, None,
                        op0=ALU.mult, op1=ALU.max,
                        accum_out=stat[:, 2 * j + 1:2 * j + 2])

            def S2_mlp_ca(i):
                rep, b = divmod(i, B_LOC)
                stat, hs, xbf = stat_of[rep], hs_of[rep], xbf_of[i]
                # MLP (1/HW folded into w1ta for the sum column)
                hp = mlppool.tile([MID, 2], F32, tag="mlp", name=f"hp{i}")
                for t in range(NT):
                    j = t * B_LOC + b
                    nc.tensor.matmul(hp[:, 0:1], w1ta[:, t, :],
                                     stat[:, 2 * j:2 * j + 1],
                                     start=(t == 0), stop=(t == NT - 1))
                for t in range(NT):
                    j = t * B_LOC + b
                    nc.tensor.matmul(hp[:, 1:2], w1t[:, t, :],
                                     stat[:, 2 * j + 1:2 * j + 2],
                                     start=(t == 0), stop=(t == NT - 1))
                hsum = hs[:, 2 * B_LOC + b:2 * B_LOC + b + 1]
                nc.scalar.activation(hs[:, 2 * b:2 * b + 2], hp[:],
                                     AF.Relu, accum_out=hsum)
                for t in range(NT):
                    op = mlppool.tile([128, 1], F32, tag="mlp",
                                      name=f"op{i}_{t}")
                    nc.tensor.matmul(op[:], w2t[:, t * 128:(t + 1) * 128],
                                     hsum, start=True, stop=True)
                    cacol = stat[:, 16 + t * B_LOC + b:17 + t * B_LOC + b]
                    nc.scalar.activation(cacol, op[:], AF.Sigmoid)
                # xc = x * ca in bf16 (DVE 4x)
                for t in range(NT):
                    cacol = stat[:, 16 + t * B_LOC + b:17 + t * B_LOC + b]
                    nc.vector.tensor_scalar_mul(
                        xbf[:, t, :], xbf[:, t, :], cacol)
                # channel pair max
                xm2_of[i] = xpool.tile([128, HW], BF16, tag="xm2",
                                       name=f"xm2{i}", bufs=2)
                nc.vector.tensor_max(xm2_of[i][:], xbf[:, 0, :],
                                     xbf[:, 1, :])

            def S3_feat(i):
                rep, b = divmod(i, B_LOC)
                stat, xbf = stat_of[rep], xbf_of[i]
                meanpad, maxpad = meanpads[i % NPAD], maxpads[i % NPAD]
                feat_dx = feat_dxs[i % B_LOC]
                mnv = meanpad[:].rearrange("p (y x) -> p y x", y=PH + 1, x=PW)
                mxv = maxpad[:].rearrange("p (y x) -> p y x", y=PH + 1, x=PW)
                # channel sum (PE) -> mean plane (ACT evict)
                for jc in range(8):
                    p4 = p4pool.tile([1, 512], F32, tag="p4",
                                     name=f"p4_{i}_{jc}")
                    for t in range(NT):
                        nc.tensor.matmul(
                            p4[:], ones[:],
                            xbf[:, t, jc * 512:(jc + 1) * 512],
                            start=(t == 0), stop=(t == NT - 1))
                    dst = mnv[0:1, 3 + 8 * jc:3 + 8 * jc + 8, 3:3 + W]
                    nc.scalar.activation(
                        dst, p4[0:1, :].rearrange("p (y x) -> p y x",
                                                  y=8, x=W), AF.Copy)
                # channel max plane via partition all-reduce (Pool)
                nc.gpsimd.partition_all_reduce(
                    mxv[:, 3:67, 3:67],
                    xm2_of[i][:].rearrange("p (y x) -> p y x", y=H, x=W),
                    128, bass_isa.ReduceOp.max)
                # dx shifts -> feat_dx[14, PROW] (dy KI..6 source); max-side
                # DMAs ride the Pool SWDGE queue right behind the all-reduce
                mph, xph = meanpad[:].tensor, maxpad[:].tensor
                nc.scalar.dma_start(
                    feat_dx[0:7, :],
                    bass.AP(mph, 0, [[PROW, 1], [1, 7], [1, PROW]]))
                nc.gpsimd.dma_start(
                    feat_dx[7:14, :],
                    bass.AP(xph, 0, [[PROW, 1], [1, 7], [1, PROW]]))
                # im2col for dy 0..KI-1 straight from the planes
                # rows dy-major: row = c*7*KI + dy*7 + dx
                imc = gpool.tile([2 * 7 * KI, IROW], BF16, tag="imc",
                                 name=f"imc{i}", bufs=2 if debug_taps else 4)
                imc_of[i] = imc
                for c, ph_, eng in ((0, mph, nc.scalar), (1, xph, nc.gpsimd)):
                    for dy in range(KI):
                        eng.dma_start(
                            imc[c * 7 * KI + dy * 7:
                                c * 7 * KI + dy * 7 + 7, :],
                            bass.AP(ph_, dy * PW,
                                    [[FROW, 1], [1, 7], [1, IROW]]))

            def S4_conv(i):
                feat_dx = feat_dxs[i % B_LOC]
                fdv = feat_dx[:].rearrange("p (y x) -> p y x", y=PH, x=PW)
                imv = imc_of[i][:].rearrange("p (y x) -> p y x", y=H, x=PW)
                greps = []
                for hh in range(2):
                    grep = gpool.tile([128, HW // 2], BF16, tag="grep",
                                      name=f"grep{i}_{hh}")
                    greps.append(grep)
                    for qq in range(4):
                        q = hh * 4 + qq
                        gp = gpspool.tile([128, QF], F32, tag="gps",
                                          name=f"gps{i}_{q}")
                        y0 = q * QY
                        nc.tensor.matmul(gp[:], wc3[:],
                                         imv[:, y0:y0 + QY, 0:W],
                                         start=True, stop=False)
                        for dy in range(KI, 7):
                            nc.tensor.matmul(
                                gp[:], wcd[:, dy - KI, :],
                                fdv[:, y0 + dy:y0 + dy + QY, 0:W],
                                start=False, stop=(dy == 6))
                        nc.scalar.activation(
                            grep[:, qq * QF:(qq + 1) * QF], gp[:],
                            AF.Sigmoid)
                return greps

            def S5_gate_store(i, greps):
                b = i % B_LOC
                xbf = xbf_of[i]
                dst_d = out_d.ap()[b].rearrange(
                    "ch h w -> ch (h w)").rearrange(
                    "(t c) e -> c t e", t=NT, c=128)
                for hh in range(2):
                    sl = slice(hh * (HW // 2), (hh + 1) * (HW // 2))
                    for t in range(NT):
                        nc.vector.tensor_mul(
                            xbf[:, t, sl], xbf[:, t, sl], greps[hh][:])
                    for t in range(NT):
                        nc.sync.dma_start(dst_d[:, t:t + 1, sl],
                                          xbf[:, t:t + 1, sl])

            greps_of = {}
            stages = {0: S0_load, 1: S1_pool, 2: S2_mlp_ca, 3: S3_feat,
                      4: lambda i: greps_of.__setitem__(i, S4_conv(i))}

            def S5(i):
                if debug_taps and i == 0:
                    nc.sync.dma_start(dbg_stat_d.ap(), stat_of[0][:])
                    nc.sync.dma_start(dbg_mean_d.ap(), meanpads[0][:])
                    nc.sync.dma_start(dbg_max_d.ap(), maxpads[0][0:2, :])
                    nc.sync.dma_start(dbg_fdx_d.ap(), feat_dxs[0][:])
                    nc.sync.dma_start(dbg_imc_d.ap(), imc_of[0][:])
                    for hh in range(2):
                        nc.sync.dma_start(dbg_grep_d.ap()[:, hh],
                                          greps_of[0][hh][:])
                S5_gate_store(i, greps_of.pop(i))
                del xbf_of[i], xm2_of[i], imc_of[i]
            stages[5] = S5

            sched = ([(k, 0) for k in range(6)] if debug_taps else SCHED)
            max_skew = max(sk for _, sk in sched)
            for j in range(N + max_skew):
                for st, sk in sched:
                    if 0 <= j - sk < N:
                        stages[st](j - sk)

    nc.compile()
    return nc


def _host_consts(w1, w2, ws):
    import ml_dtypes
    bf16 = ml_dtypes.bfloat16
    ones = np.ones((128, 1), np.float32).astype(bf16)
    # w1 [MID, C] -> lhsT layout [128, NT, MID]
    w1t = np.ascontiguousarray(
        np.asarray(w1, np.float32).T.reshape(NT, 128, MID).transpose(
            1, 0, 2)).astype(np.float32)
    w1ta = (w1t / float(HW)).astype(np.float32)
    w2t = np.ascontiguousarray(np.asarray(w2, np.float32).T)
    wf = np.asarray(ws, np.float32)[0]                       # [2, 7, 7]
    # wc3 rows: (c, dy<KI, dx) ordered c*7*KI + dy*7 + dx; wcd: (c, dx) x dy
    wc3 = np.empty((2 * 7 * KI, 1), np.float32)
    wcd = np.empty((14, 7 - KI, 1), np.float32)
    for c in range(2):
        scale = 1.0 / C if c == 0 else 1.0
        for dx in range(7):
            for dy in range(7):
                v = wf[c, dy, dx] * scale
                if dy < KI:
                    wc3[c * 7 * KI + dy * 7 + dx, 0] = v
                else:
                    wcd[c * 7 + dx, dy - KI, 0] = v
    wc3 = np.repeat(wc3, 128, axis=1).astype(bf16)
    wcd = np.repeat(wcd, 128, axis=2).astype(bf16)
    return {"ones": ones, "w1t": w1t, "w1ta": w1ta, "w2t": w2t,
            "wc3": wc3, "wcd": wcd}


def make_in_maps(x, w1, w2, ws):
    import ml_dtypes
    xb = np.asarray(x, np.float32).astype(ml_dtypes.bfloat16)
    consts = _host_consts(w1, w2, ws)
    return [{"x": np.ascontiguousarray(xb[i * B_LOC:(i + 1) * B_LOC]),
             **consts} for i in range(NCORES)]


def kernel(x, w1, w2, ws):
    global _cached_nc
    from concourse.bass_utils import run_bass_kernel_spmd

    if _cached_nc is None:
        _cached_nc = _build()
    nc = _cached_nc

    in_maps = make_in_maps(x, w1, w2, ws)
    res = run_bass_kernel_spmd(nc, in_maps, core_ids=list(range(NCORES)))
    out = np.concatenate([np.asarray(res.results[i]["out"])
                          for i in range(NCORES)], axis=0)
    return out.astype(np.float32)
